# revision 56
# baseline (speedup 1.0000x reference)
"""InterfaceBoundaryLoss Trainium2 kernel.

Data-parallel over batch across 8 NeuronCores.  The [H,W] interface mask is
analyzed on the host and covered with variable-height "segments" (tall thin
ones along the near-vertical arcs, short wide ones near the circle's
top/bottom vertices).  Segments are packed into the 128 SBUF partitions in
"groups" sharing a uniform column width, so every engine instruction runs
at full partition occupancy while the free dim stays small.

Per masked cell (i,j) with m=1:
  pot += (phi1-phi2)^2
  der += (EPS1*d1 - EPS2*d2)^2,  dk = nx*dpx_k + ny*dpy_k
With psi = 0.025*phi2 - phi1,  EPS1*d1 - EPS2*d2 = -40000*(nx*Dx(psi) +
ny*Dy(psi)), so with host fields A = 40000*m*nx, B = 40000*m*ny,
  der = sum((A*Dx(psi) + B*Dy(psi))^2)
Dy is computed on the TensorEngine via a per-group block-banded stationary
matrix (one +/-1 band per packed segment), Dx on the VectorEngine via
shifted views.  The pot path runs on GpSimd.  Both quadratic terms are
packed side by side in one tile and reduced with a single Square+accum on
the ScalarEngine per group.

phi1/phi2 are interleaved on the host as [2, BPC, H, W] (field-major) so one
3D DMA per segment fetches all 8 local batches of both fields (the
field*batch axis is a single uniform-stride dim) and the two field views
stay 2D-contiguous in SBUF.  DMA dispatch serializes on the shared HWDGE
unit (~630ns each, issued alternately from the Sync and Scalar sequencers),
so the planner trades segment count against free-dim size explicitly.

Host sums per-partition partials in float64; mask cells on the frame border
(edge-padding semantics) or uncovered by segments are computed exactly on
the host (none for the reference circle mask).
"""

import sys

for _p in ("/opt/trn_rl_repo",):
    if _p not in sys.path:
        sys.path.append(_p)

import numpy as np
import ml_dtypes

B, H, W = 64, 1024, 1024
EPS1, EPS2 = 80.0, 2.0
DX, DY = 0.001, 0.001
CX, CY = 512.0, 512.0
WEIGHT = 1.0
N_CORES = 8
BPC = B // N_CORES
NBF = 2 * BPC  # batch*field blocks per partition row

# planner cost model: ns per DMA dispatch vs ns per free-dim column
PLAN_DISP = 750.0
PLAN_ENG = 6.1
CHUNK_W = 59  # max owned cols per segment -> w <= 64, 8w <= 512 (one PSUM bank)
HEIGHTS = (126, 62, 30, 14)
CLUSTER_GAP = 8
SGN = 2  # towers per super-group
DMA_CHUNK_BYTES = 80000  # split segment DMAs above this size across queues

TRACE = False
LAST_EXEC_NS = None


def _normals(h, w):
    ii = np.arange(h, dtype=np.float64)[:, None]
    jj = np.arange(w, dtype=np.float64)[None, :]
    nx = jj - CX
    ny = ii - CY
    norm = np.sqrt(nx * nx + ny * ny)
    safe = np.where(norm > 0, norm, 1.0)
    return nx / safe, ny / safe


def _cluster(cols, gap):
    out = []
    s = p = cols[0]
    for c in cols[1:]:
        if c - p > gap:
            out.append((s, p))
            s = c
        p = c
    out.append((s, p))
    return out


class _Seg:
    __slots__ = ("r0", "h", "ca", "ce", "c0", "p0", "owned")

    def __init__(self, r0, h, ca, ce):
        self.r0, self.h, self.ca, self.ce = int(r0), int(h), int(ca), int(ce)

    @property
    def prows(self):
        return self.h + 2


class _Group:
    def __init__(self):
        self.segs = []
        self._p = 0
        self.w = 0
        self.coff = 0


def _plan(mask):
    """Cover interior mask cells with variable-height segments, pack into
    128-partition groups of uniform width."""
    h_, w_ = mask.shape
    border = np.zeros_like(mask)
    border[0, :] = border[-1, :] = True
    border[:, 0] = border[:, -1] = True
    host_cells = mask & border
    core = mask & ~border

    rows_any = np.flatnonzero(core.any(axis=1))
    if len(rows_any) == 0:
        return [], host_cells

    minr, maxr = int(rows_any[0]), int(rows_any[-1])

    def band_segments(r0, hh):
        band = core[r0 : r0 + hh]
        cols = np.flatnonzero(band.any(axis=0))
        if len(cols) == 0:
            return []
        segs = []
        for ca, cb in _cluster(cols, CLUSTER_GAP):
            n = cb - ca + 1
            nch = -(-n // CHUNK_W)
            step = -(-n // nch)
            for k in range(nch):
                a = ca + k * step
                e = min(ca + (k + 1) * step - 1, cb)
                segs.append(_Seg(r0, hh, a, e))
        return segs

    def seg_cost(s):
        w = -(-(s.ce - s.ca + 1 + 5) // 8) * 8
        return PLAN_DISP + PLAN_ENG * NBF // 2 * w * (s.h + 2) / 128.0

    memo = {}

    def dp(r):
        if r > maxr:
            return (0.0, ())
        if r in memo:
            return memo[r]
        best = None
        for hh in HEIGHTS:
            he = min(hh, maxr + 1 - r)
            segs = band_segments(r, he)
            c = sum(seg_cost(s) for s in segs)
            sub, subsegs = dp(r + he)
            tot = c + sub
            if best is None or tot < best[0]:
                best = (tot, tuple(segs) + subsegs)
        memo[r] = best
        return best

    _, segs = dp(minr)
    segs = list(segs)

    # pack into towers: widest first, greedy partition fill
    segs.sort(key=lambda s: -(s.ce - s.ca))
    towers = []
    for s in segs:
        for t in towers:
            if t._p + s.prows <= 128:
                t.segs.append(s)
                t._p += s.prows
                break
        else:
            t = _Group()
            t._p = s.prows
            t.segs.append(s)
            towers.append(t)

    # bin towers into super-groups of <= SGN, uniform (max) width
    towers.sort(
        key=lambda t: -max(-(-(s.ce - s.ca + 1 + 5) // 8) * 8 for s in t.segs)
    )
    sgs = [towers[i : i + SGN] for i in range(0, len(towers), SGN)]
    if len(sgs) > 2:
        # 2nd-smallest SG first: its few small DMAs gate the first compute
        sgs = [sgs[-2]] + sgs[:-2] + [sgs[-1]]

    # per tower: width = SG max, per-seg c0/p0, owned cells (dedup)
    assigned = np.zeros_like(mask)
    for sg in sgs:
        w = max(
            -(-(s.ce - s.ca + 1 + 5) // 8) * 8 for t in sg for s in t.segs
        )
        for t in sg:
            t.w = w
            p0 = 0
            for s in t.segs:
                c0 = s.ca - 2
                if c0 % 2:
                    c0 -= 1
                c0 = max(0, min(c0, w_ - w))
                s.c0 = c0
                s.p0 = p0
                p0 += s.prows
                ce = min(s.ce, c0 + w - 3)  # owned >=2 cols from window edge
                own = np.zeros((s.prows, w), dtype=bool)
                sub = (
                    core[s.r0 : s.r0 + s.h, s.ca : ce + 1]
                    & ~assigned[s.r0 : s.r0 + s.h, s.ca : ce + 1]
                )
                own[1 : 1 + s.h, s.ca - c0 : ce + 1 - c0] = sub
                assigned[s.r0 : s.r0 + s.h, s.ca : ce + 1] |= sub
                s.owned = own

    leftover = core & ~assigned
    if leftover.any():
        host_cells = host_cells | leftover
        for sg in sgs:
            for t in sg:
                for s in t.segs:
                    lv = leftover[
                        s.r0 - 1 : s.r0 - 1 + s.prows, s.c0 : s.c0 + t.w
                    ]
                    s.owned &= ~lv
    return sgs, host_cells


def _host_contrib(cells_ij, phi1, phi2, nx, ny):
    if len(cells_ij[0]) == 0:
        return 0.0, 0.0
    ii, jj = cells_ij
    p1 = phi1.astype(np.float64)
    p2 = phi2.astype(np.float64)
    d = p1[:, ii, jj] - p2[:, ii, jj]
    pot = float(np.sum(d * d))
    jc = np.clip(jj, 1, W - 2)
    ic = np.clip(ii, 1, H - 2)

    def dn(p):
        dpx = (p[:, ii, jc + 1] - p[:, ii, jc - 1]) / (2.0 * DX)
        dpy = (p[:, ic + 1, jj] - p[:, ic - 1, jj]) / (2.0 * DY)
        return nx[ii, jj] * dpx + ny[ii, jj] * dpy

    mm = EPS1 * dn(p1) - EPS2 * dn(p2)
    der = float(np.sum(mm * mm))
    return pot, der


def _prepare(mask):
    nx, ny = _normals(H, W)
    sgs, host_cells = _plan(mask)
    np_dt = ml_dtypes.bfloat16

    af = 40000.0 * nx
    bf = 40000.0 * ny

    ntw = sum(len(sg) for sg in sgs)
    ctot = sum(3 * len(sg) * sg[0].w for sg in sgs)
    cst = np.zeros((128, ctot), dtype=np.float64)
    # tower blocks 0..ntw-1: banded Dy matrices; block ntw: -I; ntw+1: +I
    dmats = np.zeros((128, 128 * (ntw + 2)), dtype=np.float64)
    for p in range(128):
        dmats[p, 128 * ntw + p] = -1.0
        dmats[p, 128 * (ntw + 1) + p] = 1.0
    coff = 0
    ti = 0
    for sg in sgs:
        w = sg[0].w
        T = len(sg)
        for t_i, t in enumerate(sg):
            for s in t.segs:
                rs = slice(s.r0 - 1, s.r0 - 1 + s.prows)
                cs = slice(s.c0, s.c0 + w)
                a_box = np.where(s.owned, af[rs, cs], 0.0)
                b_box = np.where(s.owned, bf[rs, cs], 0.0)
                ps = slice(s.p0, s.p0 + s.prows)
                ao = coff + t_i * w
                bo = coff + (T + t_i) * w
                mo = coff + (2 * T + t_i) * w
                # pre-shift A/B left one col: field[f] = value at col f+1
                cst[ps, ao : ao + w - 1] = a_box[:, 1:]
                cst[ps, bo : bo + w - 1] = b_box[:, 1:]
                cst[ps, mo : mo + w] = s.owned
                # dmat block: dy[p] = psi[p+1] - psi[p-1] on interior rows
                for lr in range(1, s.h + 1):
                    p = s.p0 + lr
                    dmats[p + 1, ti * 128 + p] = 1.0
                    dmats[p - 1, ti * 128 + p] = -1.0
            ti += 1
        coff += 3 * T * w

    consts = {
        "cst": cst.astype(np_dt),
        "dmats": dmats.astype(np_dt),
    }
    return sgs, consts, host_cells, np_dt


def _build_nc(sgs, ctot):
    from contextlib import ExitStack
    from concourse import bass, bacc, tile, mybir

    mdt = mybir.dt.bfloat16
    f32 = mybir.dt.float32
    mult = mybir.AluOpType.mult
    sub = mybir.AluOpType.subtract
    SQ = mybir.ActivationFunctionType.Square

    nsg = len(sgs)
    ntw = sum(len(sg) for sg in sgs)
    nc = bacc.Bacc(
        "TRN2", target_bir_lowering=False, debug=False, num_devices=N_CORES
    )

    pf_d = nc.dram_tensor("pf", [2 * BPC * H, W], mdt, kind="ExternalInput")
    cst_d = nc.dram_tensor("cst", [128, ctot], mdt, kind="ExternalInput")
    dm_d = nc.dram_tensor(
        "dmats", [128, 128 * (ntw + 2)], mdt, kind="ExternalInput"
    )
    acc_d = nc.dram_tensor("acc", [128, 2 * nsg], f32, kind="ExternalOutput")

    with tile.TileContext(nc) as tc, ExitStack() as ctx:
        io = ctx.enter_context(tc.tile_pool(name="io", bufs=2))
        mid = ctx.enter_context(tc.tile_pool(name="mid", bufs=2))
        psum = ctx.enter_context(tc.tile_pool(name="psum", bufs=3, space="PSUM"))
        zp = ctx.enter_context(tc.tile_pool(name="zp", bufs=3))
        onep = ctx.enter_context(tc.tile_pool(name="onep", bufs=1))

        cstt = onep.tile([128, ctot], mdt)
        nc.scalar.dma_start(cstt[:], cst_d.ap())
        dmt = onep.tile([128, 128 * (ntw + 2)], mdt)
        nc.scalar.dma_start(dmt[:], dm_d.ap())
        acc = onep.tile([128, 2 * nsg], f32)
        nc.vector.memset(acc[:], 0.0)

        coff = 0
        ti = 0
        ndma = 0
        for gi, sg in enumerate(sgs):
            w = sg[0].w
            T = len(sg)
            x = BPC * w  # per-tower der width
            F = T * x
            co = coff
            coff += 3 * T * w

            ft = io.tile([128, 2 * F], mdt, tag="ft")
            for t_i, t in enumerate(sg):
                P_t = sum(s.prows for s in t.segs)
                if P_t < 128:
                    pa = (P_t // 32) * 32  # partition base must be 32-aligned
                    nc.vector.memset(
                        ft[pa:128, t_i * 2 * x : (t_i + 1) * 2 * x], 0.0
                    )
                for s in t.segs:
                    # each DMA serializes on ONE of the 16 queues at
                    # ~23GB/s; chunk the FIRST super-group's segments so its
                    # transfers parallelize (it gates the first compute) --
                    # later groups overlap with compute anyway and extra
                    # dispatches would delay them
                    rmax = 128
                    r = 0
                    while r < s.prows:
                        rn = min(rmax, s.prows - r)
                        src = bass.AP(
                            pf_d,
                            (s.r0 - 1 + r) * W + s.c0,
                            [[W, rn], [H * W, NBF], [1, w]],
                        )
                        dst = ft[
                            s.p0 + r : s.p0 + r + rn,
                            t_i * 2 * x : (t_i + 1) * 2 * x,
                        ].rearrange("p (q w) -> p q w", q=NBF)
                        # alternate HWDGE dispatchers (2:1 toward sync --
                        # the scalar sequencer also runs the Square ops)
                        eng = nc.sync if ndma % 3 < 2 else nc.scalar
                        eng.dma_start(dst, src)
                        ndma += 1
                        r += rn

            ftv = ft[:].rearrange("p (t f x) -> p t f x", t=T, f=2)
            f1v = ftv[:, :, 0, :]
            f2v = ftv[:, :, 1, :]

            # psi = 0.025*phi2 - phi1 (whole super-group)
            psi = mid.tile([128, F], mdt, tag="psi")
            nc.vector.scalar_tensor_tensor(
                psi[:].rearrange("p (t x) -> p t x", t=T),
                f2v,
                0.025,
                f1v,
                op0=mult,
                op1=sub,
            )

            # dxs[f] = psi[f+2] - psi[f]  (cell at f+1)
            dxs = mid.tile([128, F], mdt, tag="dxs")
            nc.vector.tensor_sub(
                dxs[:, 0 : F - 2], psi[:, 2:F], psi[:, 0 : F - 2]
            )
            nc.vector.memset(dxs[:, F - 2 : F], 0.0)

            # u = A * dxs, single 4D-view mul
            a4 = (
                cstt[:, co : co + T * w]
                .rearrange("p (t w) -> p t w", t=T)
                .unsqueeze(2)
                .broadcast_to([128, T, BPC, w])
            )
            u = mid.tile([128, F], mdt, tag="u")
            nc.vector.tensor_mul(
                u[:].rearrange("p (t b w) -> p t b w", t=T, b=BPC),
                dxs[:].rearrange("p (t b w) -> p t b w", t=T, b=BPC),
                a4,
            )

            # dy + v per tower
            v = mid.tile([128, F], mdt, tag="v")
            for t_i, t in enumerate(sg):
                P_t = sum(s.prows for s in t.segs)
                dy = psum.tile([128, 512], f32, tag="dy")
                nc.tensor.matmul(
                    dy[:, 0:x],
                    dmt[0:P_t, (ti + t_i) * 128 : (ti + t_i) * 128 + 128],
                    psi[0:P_t, t_i * x : (t_i + 1) * x],
                    start=True,
                    stop=True,
                )
                dy3 = dy[:, 0:x].rearrange("p (b w) -> p b w", b=BPC)
                b3 = (
                    cstt[:, co + (T + t_i) * w : co + (T + t_i + 1) * w]
                    .unsqueeze(1)
                    .broadcast_to([128, BPC, w])
                )
                v3 = v[:, t_i * x : (t_i + 1) * x].rearrange(
                    "p (b w) -> p b w", b=BPC
                )
                nc.vector.memset(v3[:, :, w - 1 : w], 0.0)
                nc.vector.tensor_mul(
                    v3[:, :, 0 : w - 1], b3[:, :, 0 : w - 1], dy3[:, :, 1:w]
                )
            ti += T

            z = zp.tile([128, 2 * F], mdt, tag="z")
            nc.gpsimd.tensor_add(z[:, 0:F], u[:, :], v[:, :])

            # pot path: df = phi2-phi1 on the PE (accumulating -I/+I
            # matmuls), mask-mul on GpSimd is illegal (no PSUM access) so
            # it runs on Vector
            for t_i, t in enumerate(sg):
                P_t = sum(s.prows for s in t.segs)
                dfp = psum.tile([128, 512], f32, tag="dfp")
                nc.tensor.matmul(
                    dfp[:, 0:x],
                    dmt[0:P_t, ntw * 128 : ntw * 128 + 128],
                    ft[0:P_t, t_i * 2 * x : t_i * 2 * x + x],
                    start=True,
                    stop=False,
                )
                nc.tensor.matmul(
                    dfp[:, 0:x],
                    dmt[0:P_t, (ntw + 1) * 128 : (ntw + 1) * 128 + 128],
                    ft[0:P_t, t_i * 2 * x + x : (t_i + 1) * 2 * x],
                    start=False,
                    stop=True,
                )
                m3 = (
                    cstt[:, co + (2 * T + t_i) * w : co + (2 * T + t_i + 1) * w]
                    .unsqueeze(1)
                    .broadcast_to([128, BPC, w])
                )
                nc.vector.tensor_mul(
                    z[:, F + t_i * x : F + (t_i + 1) * x].rearrange(
                        "p (b w) -> p b w", b=BPC
                    ),
                    dfp[:, 0:x].rearrange("p (b w) -> p b w", b=BPC),
                    m3,
                )

            # split Square: each half reduces as soon as its producer
            # (V mask-muls for pot, G add for der) finishes
            zsq = zp.tile([128, 2 * F], mdt, tag="zsq")
            nc.scalar.activation(
                zsq[:, F : 2 * F],
                z[:, F : 2 * F],
                SQ,
                accum_out=acc[:, 2 * gi + 1 : 2 * gi + 2],
            )
            nc.scalar.activation(
                zsq[:, 0:F],
                z[:, 0:F],
                SQ,
                accum_out=acc[:, 2 * gi : 2 * gi + 1],
            )

        nc.sync.dma_start(acc_d.ap(), acc[:])

    nc.compile()
    return nc


_CACHE = {}


def kernel(output_in, output_out, interface_mask):
    from concourse.bass_utils import run_bass_kernel_spmd

    phi1 = np.asarray(output_in).reshape(B, H, W)
    phi2 = np.asarray(output_out).reshape(B, H, W)
    mask = np.asarray(interface_mask).astype(bool)

    n_mask = float(mask.sum())
    if n_mask == 0.0:
        return np.float32(np.nan)

    key = mask.tobytes()
    if key not in _CACHE:
        sgs, consts, host_cells, np_dt = _prepare(mask)
        ctot = sum(3 * len(sg) * sg[0].w for sg in sgs)
        nc = _build_nc(sgs, ctot) if sgs else None
        _CACHE[key] = (sgs, consts, host_cells, np_dt, nc)
    sgs, consts, host_cells, np_dt, nc = _CACHE[key]

    pot = der = 0.0
    if nc is not None:
        in_maps = []
        for c in range(N_CORES):
            sl = slice(c * BPC, (c + 1) * BPC)
            m = dict(consts)
            pf = np.stack(
                [phi1[sl], phi2[sl]], axis=0
            )  # [2, BPC, H, W] field-major
            m["pf"] = pf.reshape(2 * BPC * H, W).astype(np_dt)
            in_maps.append(m)
        res = run_bass_kernel_spmd(
            nc, in_maps, core_ids=list(range(N_CORES)), trace=TRACE
        )
        global LAST_EXEC_NS
        LAST_EXEC_NS = res.exec_time_ns
        for r in res.results:
            a = r["acc"].astype(np.float64)
            both = float(a.sum())
            pot += 0.0
            der += both  # pot+der combined in one accumulator

    if host_cells.any():
        nx, ny = _normals(H, W)
        hp, hd = _host_contrib(np.nonzero(host_cells), phi1, phi2, nx, ny)
        pot += hp
        der += hd

    denom = B * n_mask
    return np.float32(WEIGHT * (pot + der) / denom)
